# revision 3
# baseline (speedup 1.0000x reference)
"""Trainium2 Bass kernel for nn_ExpertGroup (moe_routing).

Sharding: 8 cores = (batch b in 0..3) x (seq half j in 0..1); each core owns
1024 tokens. Activations flow in transposed [feature, token] layout so every
matmul contracts over the partition dim. The sequence-mixing adapt attention
needs full-S adapt_in/adapt_out, so paired cores AllGather their N-layout
halves (one collective carrying both tensors), overlapped with the
independent expert branch. Matmul operands are bf16 (host-cast weights),
accumulation f32 in PSUM.
"""

import numpy as np
import ml_dtypes

import concourse.bacc as bacc
import concourse.mybir as mybir
import concourse.tile as tile
from concourse import bass_utils

F32 = mybir.dt.float32
BF16 = mybir.dt.bfloat16
AX = mybir.AxisListType
OP = mybir.AluOpType
AF = mybir.ActivationFunctionType

B, S, D, H, AD, E = 4, 2048, 1024, 2048, 128, 8
TOK = 1024          # tokens per core
N_CORES = 8
NCH = TOK // 512    # 512-wide matmul chunks of the own token range
BF = ml_dtypes.bfloat16

_NC_CACHE = None


def build():
    nc = bacc.Bacc("TRN2", target_bir_lowering=False, debug=False,
                   num_devices=N_CORES)

    # ---- per-core DRAM parameters ----
    xt = nc.declare_dram_parameter("xt", [D, TOK], BF16, isOutput=False)
    ew = nc.declare_dram_parameter("ew", [TOK, E], F32, isOutput=False)
    ewt = nc.declare_dram_parameter("ewt", [E, TOK], F32, isOutput=False)
    wu_t = nc.declare_dram_parameter("wu_t", [D, H], BF16, isOutput=False)
    wg_t = nc.declare_dram_parameter("wg_t", [D, H], BF16, isOutput=False)
    wd_t = nc.declare_dram_parameter("wd_t", [H, D], BF16, isOutput=False)
    wo_t = nc.declare_dram_parameter("wo_t", [H, D], BF16, isOutput=False)
    wpre_t = nc.declare_dram_parameter("wpre_t", [D, AD], BF16, isOutput=False)
    wpost_t = nc.declare_dram_parameter("wpost_t", [H, AD], BF16, isOutput=False)
    wap_t = nc.declare_dram_parameter("wap_t", [AD, H], BF16, isOutput=False)  # 0.1 folded
    wp = nc.declare_dram_parameter("wp", [H, AD], BF16, isOutput=False)        # 0.1 folded
    a_t = nc.declare_dram_parameter("a_t", [E, AD, AD], BF16, isOutput=False)
    bu = nc.declare_dram_parameter("bu", [H], F32, isOutput=False)
    bg = nc.declare_dram_parameter("bg", [H], F32, isOutput=False)
    bd = nc.declare_dram_parameter("bd", [D], F32, isOutput=False)
    bpre = nc.declare_dram_parameter("bpre", [AD], F32, isOutput=False)
    bpost = nc.declare_dram_parameter("bpost", [AD], F32, isOutput=False)
    ln_g = nc.declare_dram_parameter("ln_g", [AD], F32, isOutput=False)
    ln_b = nc.declare_dram_parameter("ln_b", [AD], F32, isOutput=False)
    eg = nc.declare_dram_parameter("eg", [E, AD], F32, isOutput=False)
    eb = nc.declare_dram_parameter("eb", [E, AD], F32, isOutput=False)
    id_bf = nc.declare_dram_parameter("id_bf", [128, 128], BF16, isOutput=False)
    id_f32 = nc.declare_dram_parameter("id_f32", [128, 128], F32, isOutput=False)
    out = nc.declare_dram_parameter("out", [D, TOK], F32, isOutput=True)

    with tile.TileContext(nc) as tc:
        _emit(nc, tc, locals())
    nc.compile()
    return nc


def _emit(nc, tc, P):
    xt, ew, ewt = P["xt"], P["ew"], P["ewt"]
    wu_t, wg_t, wd_t, wo_t = P["wu_t"], P["wg_t"], P["wd_t"], P["wo_t"]
    wpre_t, wpost_t, wap_t, wp, a_t = (
        P["wpre_t"], P["wpost_t"], P["wap_t"], P["wp"], P["a_t"])
    bu, bg, bd, bpre, bpost = P["bu"], P["bg"], P["bd"], P["bpre"], P["bpost"]
    ln_g, ln_b, eg, eb = P["ln_g"], P["ln_b"], P["eg"], P["eb"]
    id_bf, id_f32, out = P["id_bf"], P["id_f32"], P["out"]

    ctx = tc  # alias

    import contextlib
    stack = contextlib.ExitStack()
    pool = stack.enter_context(tc.tile_pool(name="res", bufs=1))
    scr = stack.enter_context(tc.tile_pool(name="scr", bufs=2))
    wpool = stack.enter_context(tc.tile_pool(name="wts", bufs=2))
    ps = stack.enter_context(tc.tile_pool(name="ps", bufs=2, space="PSUM"))
    dram = stack.enter_context(tc.tile_pool(name="dram", bufs=1, space="DRAM"))

    # =================== P0: constants / small prep ===================
    ident_b = pool.tile([128, 128], BF16, tag="ident_b")
    ident_f = pool.tile([128, 128], F32, tag="ident_f")
    nc.sync.dma_start(ident_b[:], id_bf[:])
    nc.sync.dma_start(ident_f[:], id_f32[:])

    but = pool.tile([128, 16], F32, tag="but")
    bgt = pool.tile([128, 16], F32, tag="bgt")
    bdt = pool.tile([128, 8], F32, tag="bdt")
    nc.sync.dma_start(but[:], bu.ap().rearrange("(t p) -> p t", p=128))
    nc.sync.dma_start(bgt[:], bg.ap().rearrange("(t p) -> p t", p=128))
    nc.sync.dma_start(bdt[:], bd.ap().rearrange("(t p) -> p t", p=128))
    bpre_c = pool.tile([128, 1], F32, tag="bpre_c")
    bpost_c = pool.tile([128, 1], F32, tag="bpost_c")
    nc.sync.dma_start(bpre_c[:], bpre.ap().unsqueeze(1))
    nc.sync.dma_start(bpost_c[:], bpost.ap().unsqueeze(1))

    lngr = pool.tile([1, 128], F32, tag="lngr")
    lnbr = pool.tile([1, 128], F32, tag="lnbr")
    nc.sync.dma_start(lngr[:], ln_g.ap().unsqueeze(0))
    nc.sync.dma_start(lnbr[:], ln_b.ap().unsqueeze(0))
    gB = pool.tile([128, 128], F32, tag="gB")
    bB = pool.tile([128, 128], F32, tag="bB")
    nc.gpsimd.partition_broadcast(gB[:], lngr[:])
    nc.gpsimd.partition_broadcast(bB[:], lnbr[:])

    egr = pool.tile([1, E * AD], F32, tag="egr")
    ebr = pool.tile([1, E * AD], F32, tag="ebr")
    nc.sync.dma_start(egr[:], eg.ap().rearrange("e a -> (e a)").unsqueeze(0))
    nc.sync.dma_start(ebr[:], eb.ap().rearrange("e a -> (e a)").unsqueeze(0))
    egB = pool.tile([128, E, AD], F32, tag="egB")
    ebB = pool.tile([128, E, AD], F32, tag="ebB")
    for e in range(E):
        nc.gpsimd.partition_broadcast(egB[:, e, :], egr[:, e * AD:(e + 1) * AD])
        nc.gpsimd.partition_broadcast(ebB[:, e, :], ebr[:, e * AD:(e + 1) * AD])

    ew_sb = pool.tile([128, 8, E], F32, tag="ew_sb")
    nc.sync.dma_start(ew_sb[:], ew.ap().rearrange("(t p) e -> p t e", p=128))
    ewr = pool.tile([128, 8, E], F32, tag="ewr")
    nc.vector.tensor_scalar_max(ewr[:], ew_sb[:], 0.0)

    ewt_sb = pool.tile([E, TOK], F32, tag="ewt_sb")
    nc.sync.dma_start(ewt_sb[:], ewt[:])
    ones8 = pool.tile([E, 1], F32, tag="ones8")
    nc.vector.memset(ones8[:], 1.0)
    sumw_row = pool.tile([1, TOK], F32, tag="sumw_row")
    for n in range(NCH):
        psw = ps.tile([1, 512], F32, tag="ps")
        nc.tensor.matmul(psw[:], ones8[:], ewt_sb[:, n * 512:(n + 1) * 512],
                         start=True, stop=True)
        nc.vector.tensor_copy(sumw_row[:, n * 512:(n + 1) * 512], psw[:])
    sumwB = pool.tile([128, TOK], F32, tag="sumwB")
    nc.gpsimd.partition_broadcast(sumwB[:], sumw_row[:])

    # stationary weight banks
    wpre_sb = pool.tile([128, 8, AD], BF16, tag="wpre_sb")
    nc.sync.dma_start(wpre_sb[:], wpre_t.ap().rearrange("(k p) a -> p k a", p=128))
    wpost_sb = pool.tile([128, 16, AD], BF16, tag="wpost_sb")
    nc.sync.dma_start(wpost_sb[:], wpost_t.ap().rearrange("(k p) a -> p k a", p=128))
    wap_sb = pool.tile([128, 16, 128], BF16, tag="wap_sb")
    nc.sync.dma_start(wap_sb[:], wap_t.ap().rearrange("a (k h) -> a k h", h=128))
    wp_sb = pool.tile([128, 16, AD], BF16, tag="wp_sb")
    nc.sync.dma_start(wp_sb[:], wp.ap().rearrange("(k p) a -> p k a", p=128))
    at_sb = pool.tile([128, E, AD], BF16, tag="at_sb")
    nc.sync.dma_start(at_sb[:], a_t.ap().rearrange("e a c -> a e c"))

    xt_sb = pool.tile([128, 8, TOK], BF16, tag="xt_sb")
    nc.sync.dma_start(xt_sb[:], xt.ap().rearrange("(k p) s -> p k s", p=128))

    # ---- LN helper (N-layout [128 tok, nb, 128 ad] blocks) ----
    def layer_norm(src, nb, dst, tag):
        """dst[:, i, :] = LN(src[:, i, :]) * gB + bB, src/dst [128, nb, 128]."""
        red = scr.tile([128, nb], F32, tag=tag + "_red")
        nc.vector.tensor_reduce(red[:], src[:], AX.X, OP.add)
        sq = scr.tile([128, nb, 128], F32, tag=tag + "_sq")
        nc.scalar.activation(sq[:], src[:], AF.Square)
        red2 = scr.tile([128, nb], F32, tag=tag + "_red2")
        nc.vector.tensor_reduce(red2[:], sq[:], AX.X, OP.add)
        m = scr.tile([128, nb], F32, tag=tag + "_m")
        nc.vector.tensor_scalar_mul(m[:], red[:], 1.0 / AD)
        msq = scr.tile([128, nb], F32, tag=tag + "_msq")
        nc.vector.tensor_tensor(msq[:], m[:], m[:], OP.mult)
        v = scr.tile([128, nb], F32, tag=tag + "_v")
        nc.vector.tensor_scalar(v[:], red2[:], 1.0 / AD, 1e-5, OP.mult, OP.add)
        v2 = scr.tile([128, nb], F32, tag=tag + "_v2")
        nc.vector.tensor_tensor(v2[:], v[:], msq[:], OP.subtract)
        sd = scr.tile([128, nb], F32, tag=tag + "_sd")
        nc.scalar.sqrt(sd[:], v2[:])
        rs = scr.tile([128, nb], F32, tag=tag + "_rs")
        nc.vector.reciprocal(rs[:], sd[:])
        for i in range(nb):
            nrm = scr.tile([128, 128], F32, tag=tag + "_nrm")
            nc.vector.tensor_scalar(nrm[:], src[:, i, :], m[:, i:i + 1],
                                    rs[:, i:i + 1], OP.subtract, OP.mult)
            nrm2 = scr.tile([128, 128], F32, tag=tag + "_nrm2")
            nc.vector.tensor_tensor(nrm2[:], nrm[:], gB[:], OP.mult)
            nc.vector.tensor_tensor(dst[:, i, :], nrm2[:], bB[:], OP.add)

    def transpose_blk(dst, src_ap, dtype):
        """dst[128,128] sbuf slice <- src_ap.T via PE (dtype BF16 or F32)."""
        pt = ps.tile([128, 128], dtype, tag="ps")
        nc.tensor.transpose(pt[:], src_ap, ident_b[:] if dtype == BF16 else ident_f[:])
        nc.vector.tensor_copy(dst, pt[:])

    # =================== P1: pre (own tokens, T-layout) ===================
    preT = pool.tile([128, TOK], BF16, tag="preT")
    for n in range(NCH):
        pp = ps.tile([128, 512], F32, tag="ps")
        for k in range(8):
            nc.tensor.matmul(pp[:], wpre_sb[:, k, :], xt_sb[:, k, n * 512:(n + 1) * 512],
                             start=(k == 0), stop=(k == 7))
        nc.vector.tensor_scalar(preT[:, n * 512:(n + 1) * 512], pp[:],
                                bpre_c[:], None, OP.add)

    # =================== P2: adapt_in (own) ===================
    preN = pool.tile([128, 8, AD], BF16, tag="preN")
    for i in range(8):
        transpose_blk(preN[:, i, :], preT[:, i * 128:(i + 1) * 128], BF16)
    ainN = pool.tile([128, 8, AD], BF16, tag="ainN")
    layer_norm(preN, 8, ainN, "lnin")
    ainT = pool.tile([128, TOK], BF16, tag="ainT")
    for i in range(8):
        transpose_blk(ainT[:, i * 128:(i + 1) * 128], ainN[:, i, :], BF16)

    # =================== P3: up/gate -> hiddenT, wpost accum ===================
    hT = pool.tile([128, 16, TOK], BF16, tag="hT")
    ppo_pool = tc.alloc_tile_pool(name="ppo_pool", bufs=2, space="PSUM")
    ppo = [ppo_pool.tile([128, 512], F32, tag="ppo", name=f"ppo{n}") for n in range(NCH)]
    with tc.tile_pool(name="pug", bufs=2, space="PSUM") as pug:
        for ht in range(16):
            wu_ht = wpool.tile([128, 8, 128], BF16, tag="wu_ht")
            wg_ht = wpool.tile([128, 8, 128], BF16, tag="wg_ht")
            nc.sync.dma_start(
                wu_ht[:], wu_t.ap()[:, ht * 128:(ht + 1) * 128]
                .rearrange("(k p) h -> p k h", p=128))
            nc.sync.dma_start(
                wg_ht[:], wg_t.ap()[:, ht * 128:(ht + 1) * 128]
                .rearrange("(k p) h -> p k h", p=128))
            for n in range(NCH):
                pu = pug.tile([128, 512], F32, tag="pu")
                pg = pug.tile([128, 512], F32, tag="pg")
                for k in range(8):
                    nc.tensor.matmul(pu[:], wu_ht[:, k, :],
                                     xt_sb[:, k, n * 512:(n + 1) * 512],
                                     start=(k == 0), stop=(k == 7))
                for k in range(8):
                    nc.tensor.matmul(pg[:], wg_ht[:, k, :],
                                     xt_sb[:, k, n * 512:(n + 1) * 512],
                                     start=(k == 0), stop=(k == 7))
                silg = scr.tile([128, 512], F32, tag="silg")
                nc.scalar.activation(silg[:], pg[:], AF.Silu,
                                     bias=bgt[:, ht:ht + 1])
                nc.vector.scalar_tensor_tensor(
                    hT[:, ht, n * 512:(n + 1) * 512], pu[:], but[:, ht:ht + 1],
                    silg[:], OP.add, OP.mult)
                nc.tensor.matmul(ppo[n][:], wpost_sb[:, ht, :],
                                 hT[:, ht, n * 512:(n + 1) * 512],
                                 start=(ht == 0), stop=(ht == 15))

    # =================== P4a: adapt_out (own) + collective ===================
    postT = pool.tile([128, TOK], BF16, tag="postT")
    for n in range(NCH):
        nc.vector.tensor_scalar(postT[:, n * 512:(n + 1) * 512], ppo[n][:],
                                bpost_c[:], None, OP.add)
    ppo_pool.release()
    postN = pool.tile([128, 8, AD], BF16, tag="postN")
    for i in range(8):
        transpose_blk(postN[:, i, :], postT[:, i * 128:(i + 1) * 128], BF16)
    aoutN = pool.tile([128, 8, AD], BF16, tag="aoutN")
    layer_norm(postN, 8, aoutN, "lnout")

    cc_in = dram.tile([2 * TOK, AD], BF16, tag="cc_in")
    cc_out = dram.tile([4 * TOK, AD], BF16, tag="cc_out")
    nc.sync.dma_start(
        cc_in[0:TOK, :].rearrange("(t p) a -> p t a", p=128), ainN[:])
    nc.sync.dma_start(
        cc_in[TOK:2 * TOK, :].rearrange("(t p) a -> p t a", p=128), aoutN[:])
    nc.gpsimd.collective_compute(
        "AllGather", OP.bypass,
        replica_groups=[[0, 1], [2, 3], [4, 5], [6, 7]],
        ins=[cc_in[:].opt()], outs=[cc_out[:].opt()])

    # =================== P5: expert branch (independent of collective) =======
    hw = pool.tile([128, 8, AD], F32, tag="hw")
    with tc.tile_pool(name="pexp", bufs=3, space="PSUM") as pexp:
        for i in range(8):
            for e in range(E):
                ph = pexp.tile([128, AD], F32, tag="ph")
                nc.tensor.matmul(ph[:], preT[:, i * 128:(i + 1) * 128],
                                 at_sb[:, e, :], start=True, stop=True)
                ss = scr.tile([128, 1], F32, tag="e_ss")
                sqh = scr.tile([128, AD], F32, tag="e_sq")
                nc.scalar.activation(sqh[:], ph[:], AF.Square, accum_out=ss[:])
                red = scr.tile([128, 1], F32, tag="e_red")
                nc.vector.tensor_reduce(red[:], ph[:], AX.X, OP.add)
                m = scr.tile([128, 1], F32, tag="e_m")
                nc.vector.tensor_scalar_mul(m[:], red[:], 1.0 / AD)
                msq = scr.tile([128, 1], F32, tag="e_msq")
                nc.vector.tensor_tensor(msq[:], m[:], m[:], OP.mult)
                v = scr.tile([128, 1], F32, tag="e_v")
                nc.vector.tensor_scalar(v[:], ss[:], 1.0 / AD, 1e-5, OP.mult, OP.add)
                v2 = scr.tile([128, 1], F32, tag="e_v2")
                nc.vector.tensor_tensor(v2[:], v[:], msq[:], OP.subtract)
                sd = scr.tile([128, 1], F32, tag="e_sd")
                nc.scalar.sqrt(sd[:], v2[:])
                rs = scr.tile([128, 1], F32, tag="e_rs")
                nc.vector.reciprocal(rs[:], sd[:])
                nrm = scr.tile([128, AD], F32, tag="e_nrm")
                nc.vector.tensor_scalar(nrm[:], ph[:], m[:], rs[:],
                                        OP.subtract, OP.mult)
                nrm2 = scr.tile([128, AD], F32, tag="e_nrm2")
                nc.vector.tensor_tensor(nrm2[:], nrm[:], egB[:, e, :], OP.mult)
                nrm3 = scr.tile([128, AD], F32, tag="e_nrm3")
                nc.vector.tensor_tensor(nrm3[:], nrm2[:], ebB[:, e, :], OP.add)
                if e == 0:
                    nc.vector.tensor_scalar_mul(hw[:, i, :], nrm3[:],
                                                ewr[:, i, e:e + 1])
                else:
                    nc.vector.scalar_tensor_tensor(
                        hw[:, i, :], nrm3[:], ewr[:, i, e:e + 1], hw[:, i, :],
                        OP.mult, OP.add)
    hwT = pool.tile([128, TOK], BF16, tag="hwT")
    for i in range(8):
        transpose_blk(hwT[:, i * 128:(i + 1) * 128], hw[:, i, :], F32)

    # Wc = (0.1*Wo@Wp).T  [AD, D]
    wc = pool.tile([128, D], BF16, tag="wc")
    with tc.tile_pool(name="pwc", bufs=2, space="PSUM") as pwc_pool:
        for n in range(2):
            pwc = pwc_pool.tile([128, 512], F32, tag="pwc")
            for k in range(16):
                wo_k = wpool.tile([128, 512], BF16, tag="wo_k")
                nc.sync.dma_start(
                    wo_k[:], wo_t.ap()[k * 128:(k + 1) * 128,
                                       n * 512:(n + 1) * 512])
                nc.tensor.matmul(pwc[:], wp_sb[:, k, :], wo_k[:],
                                 start=(k == 0), stop=(k == 15))
            nc.vector.tensor_copy(wc[:, n * 512:(n + 1) * 512], pwc[:])

    # =================== P4b: collective readback ===================
    ainN_f = pool.tile([128, 16, AD], BF16, tag="ainN_f")
    aoutN_f = pool.tile([128, 16, AD], BF16, tag="aoutN_f")
    nc.sync.dma_start(ainN_f[:, 0:8, :],
                      cc_out[0:TOK, :].rearrange("(t p) a -> p t a", p=128))
    nc.sync.dma_start(ainN_f[:, 8:16, :],
                      cc_out[2 * TOK:3 * TOK, :].rearrange("(t p) a -> p t a", p=128))
    nc.sync.dma_start(aoutN_f[:, 0:8, :],
                      cc_out[TOK:2 * TOK, :].rearrange("(t p) a -> p t a", p=128))
    nc.sync.dma_start(aoutN_f[:, 8:16, :],
                      cc_out[3 * TOK:4 * TOK, :].rearrange("(t p) a -> p t a", p=128))
    aoutT = pool.tile([128, S], BF16, tag="aoutT")
    for t in range(16):
        transpose_blk(aoutT[:, t * 128:(t + 1) * 128], aoutN_f[:, t, :], BF16)

    # =================== P6: aw + adapt ===================
    pad_pool = tc.alloc_tile_pool(name="pad_pool", bufs=2, space="PSUM")
    pad = [pad_pool.tile([128, 512], F32, tag="pad", name=f"pad{n}") for n in range(NCH)]
    with tc.tile_pool(name="paw", bufs=3, space="PSUM") as paw_pool:
        for t in range(16):
            for n in range(NCH):
                paw = paw_pool.tile([128, 512], F32, tag="paw")
                nc.tensor.matmul(paw[:], aoutT[:, t * 128:(t + 1) * 128],
                                 ainT[:, n * 512:(n + 1) * 512],
                                 start=True, stop=True)
                cl = scr.tile([128, 512], F32, tag="cl")
                nc.vector.tensor_scalar(cl[:], paw[:], 5.0, -5.0, OP.min, OP.max)
                aw_bf = scr.tile([128, 512], BF16, tag="aw_bf")
                nc.scalar.activation(aw_bf[:], cl[:], AF.Silu)
                nc.tensor.matmul(pad[n][:], ainN_f[:, t, :], aw_bf[:],
                                 start=(t == 0), stop=(t == 15))
    adT = pool.tile([128, TOK], BF16, tag="adT")
    for n in range(NCH):
        nc.vector.tensor_copy(adT[:, n * 512:(n + 1) * 512], pad[n][:])
    pad_pool.release()

    # =================== P7: hidden += 0.1 * adapt @ Wap.T ===================
    for ht in range(16):
        for n in range(NCH):
            pwap = ps.tile([128, 512], F32, tag="ps")
            nc.tensor.matmul(pwap[:], wap_sb[:, ht, :],
                             adT[:, n * 512:(n + 1) * 512], start=True, stop=True)
            nc.vector.tensor_tensor(hT[:, ht, n * 512:(n + 1) * 512], pwap[:],
                                    hT[:, ht, n * 512:(n + 1) * 512], OP.add)

    # =================== P8: shared + combine + out ===================
    with tc.tile_pool(name="psh", bufs=2, space="PSUM") as psh_pool:
        for dt in range(8):
            wd_dt = wpool.tile([128, 16, 128], BF16, tag="wd_dt")
            nc.sync.dma_start(
                wd_dt[:], wd_t.ap()[:, dt * 128:(dt + 1) * 128]
                .rearrange("(k p) d -> p k d", p=128))
            for n in range(NCH):
                psh = psh_pool.tile([128, 512], F32, tag="psh")
                for k in range(16):
                    nc.tensor.matmul(psh[:], wd_dt[:, k, :],
                                     hT[:, k, n * 512:(n + 1) * 512],
                                     start=(k == 0), stop=(k == 15))
                pct = ps.tile([128, 512], F32, tag="ps")
                nc.tensor.matmul(pct[:], wc[:, dt * 128:(dt + 1) * 128],
                                 hwT[:, n * 512:(n + 1) * 512],
                                 start=True, stop=True)
                tcomb = scr.tile([128, 512], F32, tag="tcomb")
                nc.vector.scalar_tensor_tensor(
                    tcomb[:], psh[:], bdt[:, dt:dt + 1],
                    sumwB[:, n * 512:(n + 1) * 512], OP.add, OP.mult)
                osb = scr.tile([128, 512], F32, tag="osb")
                nc.vector.tensor_tensor(osb[:], tcomb[:], pct[:], OP.add)
                nc.sync.dma_start(
                    out.ap()[dt * 128:(dt + 1) * 128, n * 512:(n + 1) * 512],
                    osb[:])

    stack.close()


def _prep_inputs(inputs):
    f = {k: np.asarray(v, np.float32) for k, v in inputs.items()}

    def tbf(a):  # transpose + bf16, contiguous
        return np.ascontiguousarray(a.T).astype(BF)

    shared = {
        "wu_t": tbf(f["Wu"]), "wg_t": tbf(f["Wg"]), "wd_t": tbf(f["Wd"]),
        "wo_t": tbf(f["Wo"]), "wpre_t": tbf(f["Wpre"]), "wpost_t": tbf(f["Wpost"]),
        "wap_t": tbf(0.1 * f["Wap"]), "wp": (0.1 * f["Wp"]).astype(BF),
        "a_t": np.ascontiguousarray(f["A"].transpose(0, 2, 1)).astype(BF),
        "bu": f["bu"], "bg": f["bg"], "bd": f["bd"],
        "bpre": f["bpre"], "bpost": f["bpost"],
        "ln_g": f["ln_g"], "ln_b": f["ln_b"], "eg": f["eg"], "eb": f["eb"],
        "id_bf": np.eye(128, dtype=np.float32).astype(BF),
        "id_f32": np.eye(128, dtype=np.float32),
    }
    in_maps = []
    for c in range(N_CORES):
        b, j = c // 2, c % 2
        sl = slice(j * TOK, (j + 1) * TOK)
        m = dict(shared)
        m["xt"] = tbf(f["x"][b, sl, :])
        m["ew"] = np.ascontiguousarray(f["expert_weights"][b, sl, :])
        m["ewt"] = np.ascontiguousarray(f["expert_weights"][b, sl, :].T)
        in_maps.append(m)
    return in_maps


def kernel(**inputs):
    global _NC_CACHE
    if _NC_CACHE is None:
        _NC_CACHE = build()
    in_maps = _prep_inputs(inputs)
    res = bass_utils.run_bass_kernel_spmd(
        _NC_CACHE, in_maps, core_ids=list(range(N_CORES)))
    out = np.empty((B, S, D), np.float32)
    for c in range(N_CORES):
        b, j = c // 2, c % 2
        out[b, j * TOK:(j + 1) * TOK, :] = res.results[c]["out"].T
    return out
